# revision 2
# baseline (speedup 1.0000x reference)
"""Multi-head attention (SEQ=4096, d_model=1024, 16 heads of d=64) on 8 TRN2
NeuronCores, tensor-parallel over heads (2 heads/core), with an AllToAll to
re-shard from head-parallel to sequence-parallel before the output projection.

Per core c (heads 2c, 2c+1):
  1. Projections: qhT2/khT2 = [W{q,k}T_c]^T-stationary matmuls over m=d_model,
     producing head-transposed activations [128(=2x64 d), 4096] in SBUF (bf16).
     vh is produced in natural [ks, dv] layout per head, with a ones column
     appended (softmax denominator trick).
  2. Attention per head: scoresT[ks, qs] = khT^T @ qhT (K=d=64), exp on ACT
     (scale=1/temperature, no max-subtraction -- scores are ~N(0,1)), AV in
     "outT" orientation: avT[65, qs] += vh_aug^T @ PT.  Row 64 = softmax sums.
     Normalize: reciprocal of sums row, broadcast across partitions via a K=1
     matmul with a ones column, multiply -> outT[dv_local=128, qs=4096] bf16.
  3. AllToAll re-shard: send qs-chunk j to core j -> each core holds
     outT_full[dv=1024, 512 rows].  FC with full WfcT, relu + residual,
     write out rows [512c : 512c+512].

Inputs are pre-transposed/bf16-cast on the host (layout prep, untimed).
"""

import os
import sys

sys.path.insert(0, "/opt/trn_rl_repo")

import numpy as np
import ml_dtypes

import concourse.bass as bass
import concourse.mybir as mybir
import concourse.tile as tile
from concourse import bacc
from concourse.bass_utils import run_bass_kernel_spmd

# Problem constants (hardcoded per contract)
SEQ = 4096
DM = 1024
NH = 16
DK = 64
DV = 64
CORES = 8
P = 128
HL = 2 * DK  # 128: two heads' worth of head-dim per core
SROWS = SEQ // CORES  # 512 output rows per core
MO = DM // P  # 8 m-chunks of d_model
KB = SEQ // P  # 32 key blocks
F32 = mybir.dt.float32
BF16 = mybir.dt.bfloat16

# exp mode: "act" = exact exp on ScalarE; "dve" = Schraudolph bit-trick bf16 on
# VectorE; "split" = alternate tiles between the two engines.
EXP_MODE = os.environ.get("EXP_MODE", "split")
# Schraudolph constants for bf16 output bits: bits = x*EA + EB, computed in f32
# then converted to int16 and bit-viewed as bf16.  exp(x) ~= 2^(x*log2e):
# EA = 128*log2(e), EB = 127*128 - C (C tuned vs round-to-nearest conversion).
EXP_A = 128.0 / float(np.log(2.0))
EXP_B = 16256.0 - 5.5


def _exp_tile(nc, out_bf16, in_psum, scale, use_dve):
    """out = exp(scale * in), bf16 output."""
    if use_dve:
        nc.vector.tensor_scalar(
            out=out_bf16.bitcast(mybir.dt.int16),
            in0=in_psum,
            scalar1=float(scale * EXP_A),
            scalar2=float(EXP_B),
            op0=mybir.AluOpType.mult,
            op1=mybir.AluOpType.add,
        )
    else:
        nc.scalar.activation(
            out=out_bf16,
            in_=in_psum,
            func=mybir.ActivationFunctionType.Exp,
            scale=float(scale),
        )


def build(seq=SEQ, exp_mode=None):
    """Build + compile the per-core Bass program.  seq is parametrized so the
    simulator can run a shrunken version."""
    exp_mode = exp_mode or EXP_MODE
    srows = seq // CORES
    kb = seq // P
    qcs = seq // 1024 if seq >= 1024 else 1  # number of 1024-wide qs chunks
    qcw = min(1024, seq)  # qs chunk width
    nqs = qcw // 512 if qcw >= 512 else 1  # 512-wide matmul slices per chunk
    qsw = min(512, qcw)

    nc = bacc.Bacc(
        "TRN2",
        target_bir_lowering=False,
        debug=False,
        enable_asserts=True,
        num_devices=CORES,
    )

    qT = nc.dram_tensor("qT", [DM, seq], BF16, kind="ExternalInput").ap()
    kT = nc.dram_tensor("kT", [DM, seq], BF16, kind="ExternalInput").ap()
    vT = nc.dram_tensor("vT", [DM, seq], BF16, kind="ExternalInput").ap()
    wqT = nc.dram_tensor("wqT", [DM, HL], BF16, kind="ExternalInput").ap()
    wkT = nc.dram_tensor("wkT", [DM, HL], BF16, kind="ExternalInput").ap()
    wvT = nc.dram_tensor("wvT", [DM, HL], BF16, kind="ExternalInput").ap()
    wfcT = nc.dram_tensor("wfcT", [DM, DM], BF16, kind="ExternalInput").ap()
    qres = nc.dram_tensor("qres", [srows, DM], F32, kind="ExternalInput").ap()
    out = nc.dram_tensor("out", [srows, DM], F32, kind="ExternalOutput").ap()

    qT_r = qT.rearrange("(o p) s -> p o s", p=P)
    kT_r = kT.rearrange("(o p) s -> p o s", p=P)
    vT_r = vT.rearrange("(o p) s -> p o s", p=P)
    wqT_r = wqT.rearrange("(o p) h -> p o h", p=P)
    wkT_r = wkT.rearrange("(o p) h -> p o h", p=P)
    wvT_r = wvT.rearrange("(o p) h -> p o h", p=P)
    wfcT_r = wfcT.rearrange("(o p) d -> p o d", p=P)
    qres_r = qres.rearrange("(b p) d -> p b d", p=P)
    out_r = out.rearrange("(b p) d -> p b d", p=P)
    sb_blocks = srows // P  # 4

    with tile.TileContext(nc) as tc:
        with (
            tc.tile_pool(name="const", bufs=1) as cpool,
            tc.tile_pool(name="xin", bufs=8) as xpool,
            tc.tile_pool(name="pt", bufs=3) as ptpool,
            tc.tile_pool(name="small", bufs=4) as spool,
            tc.tile_pool(name="eout", bufs=3) as epool,
            tc.tile_pool(name="ps", bufs=4, space="PSUM") as ps,
            tc.tile_pool(name="dram", bufs=1, space="DRAM") as dr,
        ):
            # ---- constants / persistent tiles ----
            wq_sb = cpool.tile([P, MO, HL], BF16, tag="wq")
            wk_sb = cpool.tile([P, MO, HL], BF16, tag="wk")
            wv_sb = cpool.tile([P, MO, HL], BF16, tag="wv")
            wfc_sb = cpool.tile([P, MO, DM], BF16, tag="wfc")
            nc.sync.dma_start(wq_sb[:], wqT_r[:])
            nc.sync.dma_start(wk_sb[:], wkT_r[:])
            nc.sync.dma_start(wv_sb[:], wvT_r[:])
            nc.sync.dma_start(wfc_sb[:], wfcT_r[:])
            qres_sb = cpool.tile([P, sb_blocks, DM], F32, tag="qres")
            nc.sync.dma_start(qres_sb[:], qres_r[:])

            ones1 = cpool.tile([1, DK], F32, tag="ones1")
            nc.vector.memset(ones1[:], 1.0)

            qhT2 = cpool.tile([P, seq], BF16, tag="qhT2")
            khT2 = cpool.tile([P, seq], BF16, tag="khT2")
            # vh natural per head with ones column: [ks-block part, kb, 65]
            vh = [
                cpool.tile([P, kb, DV + 1], BF16, tag=f"vh{h}", name=f"vh{h}")
                for h in range(2)
            ]
            nc.vector.memset(vh[0][:, :, DV : DV + 1], 1.0)
            nc.vector.memset(vh[1][:, :, DV : DV + 1], 1.0)
            outT = cpool.tile([P, seq], BF16, tag="outT")

            # ---- phase 1: projections ----
            half_w = min(2048, seq)  # stream q/k/v in half-SEQ column groups
            nhalf = seq // half_w
            for src_r, w_sb, dstT in (
                (qT_r, wq_sb, qhT2),
                (kT_r, wk_sb, khT2),
            ):
                for hf in range(nhalf):
                    xts = []
                    for o in range(MO):
                        xt = xpool.tile([P, half_w], BF16, tag="xin")
                        nc.sync.dma_start(
                            xt[:], src_r[:, o, hf * half_w : (hf + 1) * half_w]
                        )
                        xts.append(xt)
                    for qc in range(half_w // qsw):
                        pp = ps.tile([P, qsw], F32, tag="ps")
                        for o in range(MO):
                            nc.tensor.matmul(
                                pp[:HL],
                                wq_sb[:, o, :] if w_sb is wq_sb else wk_sb[:, o, :],
                                xts[o][:, qc * qsw : (qc + 1) * qsw],
                                start=(o == 0),
                                stop=(o == MO - 1),
                            )
                        nc.scalar.copy(
                            out=dstT[:, hf * half_w + qc * qsw :][:, :qsw],
                            in_=pp[:HL],
                        )
            # v: natural [ks, dv] per head
            for hf in range(nhalf):
                xts = []
                for o in range(MO):
                    xt = xpool.tile([P, half_w], BF16, tag="xin")
                    nc.sync.dma_start(
                        xt[:], vT_r[:, o, hf * half_w : (hf + 1) * half_w]
                    )
                    xts.append(xt)
                for b in range(half_w // P):
                    pv = ps.tile([P, HL], F32, tag="ps")
                    for o in range(MO):
                        nc.tensor.matmul(
                            pv[:],
                            xts[o][:, b * P : (b + 1) * P],
                            wv_sb[:, o, :],
                            start=(o == 0),
                            stop=(o == MO - 1),
                        )
                    kbi = hf * (half_w // P) + b
                    nc.vector.tensor_copy(
                        out=vh[0][:, kbi, :DV], in_=pv[:, :DK]
                    )
                    nc.vector.tensor_copy(
                        out=vh[1][:, kbi, :DV], in_=pv[:, DK:HL]
                    )

            # ---- phase 2: attention ----
            exp_ctr = 0
            for h in range(2):
                hs = h * DK
                for qc in range(qcs):
                    q0 = qc * qcw
                    avT = ps.tile([DV + 1, qcw], F32, tag="ps")
                    for b in range(kb):
                        sco = ps.tile([P, qcw], F32, tag="ps")
                        for j in range(nqs):
                            nc.tensor.matmul(
                                sco[:, j * qsw : (j + 1) * qsw],
                                khT2[hs : hs + DK, b * P : (b + 1) * P],
                                qhT2[hs : hs + DK, q0 + j * qsw :][:, :qsw],
                                start=True,
                                stop=True,
                            )
                        pt = ptpool.tile([P, qcw], BF16, tag="pt")
                        if exp_mode == "split":
                            use_dve = exp_ctr % 2 == 0
                        else:
                            use_dve = exp_mode == "dve"
                        exp_ctr += 1
                        _exp_tile(nc, pt[:], sco[:], 1.0 / np.sqrt(DK), use_dve)
                        for j in range(nqs):
                            nc.tensor.matmul(
                                avT[:, j * qsw : (j + 1) * qsw],
                                vh[h][:, b, :],
                                pt[:, j * qsw : (j + 1) * qsw],
                                start=(b == 0),
                                stop=(b == kb - 1),
                            )
                    # normalize: recip of sums row, broadcast via K=1 matmul
                    rr = spool.tile([1, qcw], F32, tag="rr")
                    nc.vector.reciprocal(rr[:], avT[DV : DV + 1, :])
                    bc = ps.tile([DV, qcw], F32, tag="ps")
                    for j in range(nqs):
                        nc.tensor.matmul(
                            bc[:, j * qsw : (j + 1) * qsw],
                            ones1[:],
                            rr[:, j * qsw : (j + 1) * qsw],
                            start=True,
                            stop=True,
                        )
                    bc_sb = spool.tile([DV, qcw], F32, tag="bc")
                    nc.vector.tensor_copy(out=bc_sb[:], in_=bc[:])
                    nc.vector.tensor_mul(
                        out=outT[hs : hs + DK, q0 : q0 + qcw],
                        in0=avT[:DV, :],
                        in1=bc_sb[:],
                    )

            # ---- phase 3: AllToAll + FC ----
            a2a_in = dr.tile([CORES * P, srows], BF16)
            a2a_out = dr.tile([CORES * P, srows], BF16)
            for j in range(CORES):
                nc.sync.dma_start(
                    a2a_in[j * P : (j + 1) * P, :],
                    outT[:, j * srows : (j + 1) * srows],
                )
            nc.gpsimd.collective_compute(
                "AllToAll",
                mybir.AluOpType.bypass,
                replica_groups=[list(range(CORES))],
                ins=[a2a_in.opt()],
                outs=[a2a_out.opt()],
            )
            ofull = cpool.tile([P, MO, srows], BF16, tag="ofull")
            nc.sync.dma_start(
                ofull[:], a2a_out.rearrange("(o p) s -> p o s", p=P)
            )
            for sb in range(sb_blocks):
                for nm in range(DM // 512):
                    pf = ps.tile([P, 512], F32, tag="ps")
                    for o in range(MO):
                        nc.tensor.matmul(
                            pf[:],
                            ofull[:, o, sb * P : (sb + 1) * P],
                            wfc_sb[:, o, nm * 512 : (nm + 1) * 512],
                            start=(o == 0),
                            stop=(o == MO - 1),
                        )
                    eo = epool.tile([P, 512], F32, tag="eo")
                    nc.vector.tensor_scalar_max(out=eo[:], in0=pf[:], scalar1=0.0)
                    nc.vector.tensor_add(
                        out=eo[:],
                        in0=eo[:],
                        in1=qres_sb[:, sb, nm * 512 : (nm + 1) * 512],
                    )
                    nc.sync.dma_start(
                        out_r[:, sb, nm * 512 : (nm + 1) * 512], eo[:]
                    )

    nc.compile()
    return nc


def make_in_maps(q, k, v, Wq, Wk, Wv, Wfc, seq=SEQ):
    """Host-side layout prep + sharding."""
    srows = seq // CORES
    bf = ml_dtypes.bfloat16
    qT = np.ascontiguousarray(q.T).astype(bf)
    kT = np.ascontiguousarray(k.T).astype(bf)
    vT = np.ascontiguousarray(v.T).astype(bf)
    wfcT = np.ascontiguousarray(Wfc.T).astype(bf)
    in_maps = []
    for c in range(CORES):
        sl = slice(c * HL, (c + 1) * HL)
        in_maps.append(
            {
                "qT": qT,
                "kT": kT,
                "vT": vT,
                "wqT": np.ascontiguousarray(Wq[sl].T).astype(bf),
                "wkT": np.ascontiguousarray(Wk[sl].T).astype(bf),
                "wvT": np.ascontiguousarray(Wv[sl].T).astype(bf),
                "wfcT": wfcT,
                "qres": np.ascontiguousarray(q[c * srows : (c + 1) * srows]).astype(
                    np.float32
                ),
            }
        )
    return in_maps


_NC_CACHE = {}


def kernel(q, k, v, Wq, Wk, Wv, Wfc):
    key = "full"
    if key not in _NC_CACHE:
        _NC_CACHE[key] = build()
    nc = _NC_CACHE[key]
    in_maps = make_in_maps(q, k, v, Wq, Wk, Wv, Wfc)
    trace = bool(int(os.environ.get("KERNEL_TRACE", "0")))
    res = run_bass_kernel_spmd(
        nc, in_maps, list(range(CORES)), trace=trace
    )
    if trace:
        kernel.last_exec_time_ns = res.exec_time_ns
        kernel.last_profile = res
    out = np.concatenate([res.results[c]["out"] for c in range(CORES)], axis=0)
    return out.astype(np.float32)
